# revision 24
# baseline (speedup 1.0000x reference)
"""Trainium2 Bass kernel for nn_HarmonicOscillatorOrbitals.

out[b, i, j] = exp(-s^2/2) * H_j(s), s = omega * x[b, i, 0], j = 0..31
(physicists' Hermite polynomials), data-parallel over 8 NeuronCores on
the leading batch axis.

Per core (8192 batches = 262144 scalars as [128 partitions, E=2048]):
  env = exp(-(omega*x)^2/2), t = 2*omega*x
  G_0 = env, G_1 = t*env, G_k = t*G_{k-1} - 2(k-1)*G_{k-2}  (= env*H_k)

Engine assignment (measured on HW: DVE+GPSIMD contend on shared SBUF
ports and strictly serialize — GPSIMD is net-negative next to a busy
DVE; DVE+ACT coexist at full speed):
  DVE : whole f32 ladder, 2 ops/step (tensor_mul q = t*G_{k-1};
        scalar_tensor_tensor G_k = (G_{k-2} * -2(k-1)) + q), split in
        two column blocks; order 31 is folded straight into the fp16
        stage by the DVE (prescaled r via 2x tensor_scalar, then a
        scalar_tensor_tensor with fp16 output) so the epilogue never
        waits on an ACT cast.
  ACT : envelope, t, and the output casts: fp16 out with a per-order
        power-of-2 scale 2^(7-2k) folded into the activation-copy so
        the full f32 precision of the ladder survives the 16-bit store
        (simulated end-to-end rel err 3.3e-4 vs the 2e-2 gate).
  DMA : fp16 output = 16 MB/core, flushed per (block, 2 orders); x is
        loaded in 128 KB chunks so the first block seeds early. omega
        arrives host-replicated as [128,1] (no on-chip broadcast).

Host: descale by 2^(2k-7) while unsharding (exact power-of-2).
"""

from contextlib import ExitStack

import numpy as np

import concourse.bacc as bacc
import concourse.mybir as mybir
import concourse.tile as tile
from concourse.bass_utils import run_bass_kernel_spmd

F32 = mybir.dt.float32
F16 = mybir.dt.float16
AF = mybir.ActivationFunctionType
ALU = mybir.AluOpType

NJ = 32          # number of Hermite orders
N_CORES = 8
B = 65536        # full batch
BC = B // N_CORES
E = BC * NJ // 128   # 2048 free elems per partition per core
NB = 2           # column blocks
BW = E // NB
SEED_W = 512     # seed (Square/Exp/t) granularity
XCH = 4          # x-DMA chunks per block
FLUSH = 2        # orders per DMA flush

A_EXP = 7        # stored_k = psi_k * 2^(A_EXP - 2k); max |stored_31| ~ 48.4k


def _scale(k):
    return float(2.0 ** (A_EXP - 2 * k))


def _build():
    nc = bacc.Bacc("TRN2", target_bir_lowering=False, debug=False)
    x_d = nc.dram_tensor("x", [128, E], F32, kind="ExternalInput").ap()
    om_d = nc.dram_tensor("om", [128, 1], F32, kind="ExternalInput").ap()
    # block-major fp16 dump: [128, NB, NJ, BW] flattened
    out_d = nc.dram_tensor("out", [128, NJ * E], F16, kind="ExternalOutput").ap()

    with tile.TileContext(nc) as tc, ExitStack() as ctx:
        cpool = ctx.enter_context(tc.tile_pool(name="const", bufs=1))
        gpools = [
            ctx.enter_context(tc.tile_pool(name=f"g{b}", bufs=6)) for b in range(NB)
        ]
        qpools = [
            ctx.enter_context(tc.tile_pool(name=f"q{b}", bufs=2)) for b in range(NB)
        ]
        spools = [
            ctx.enter_context(tc.tile_pool(name=f"s{b}", bufs=3)) for b in range(NB)
        ]

        # omega rides the gpsimd software-DGE: that queue clears the NEFF
        # prologue ~1 us before the sync/scalar rings, and gpsimd is
        # otherwise idle (no SBUF-port contention while DVE is idle too)
        om2 = cpool.tile([128, 1], F32)
        nc.gpsimd.dma_start(om2[:, :], om_d[:, :])
        omneg = cpool.tile([128, 1], F32)
        nc.scalar.activation(omneg[:, :], om2[:, :], AF.Square)  # omega^2
        nc.scalar.mul(omneg[:, :], omneg[:, :], -0.5)            # -omega^2/2
        nc.scalar.mul(om2[:, :], om2[:, :], 2.0)                 # 2*omega

        x_t = cpool.tile([128, E], F32)
        t_t = cpool.tile([128, E], F32)
        sqx = cpool.tile([128, E], F32)

        # block A's x chunks split across the sync and scalar DGE rings so
        # they stream in parallel; block B's loads all go on the sync ring
        # (triggered later, under block A's seed chain) so the serial ACT
        # queue is never blocked by B's triggers
        def load_x(b):
            lo = b * BW
            for ch in range(XCH):
                w = BW // XCH
                o = lo + ch * w
                if b == 0 and ch == 0:
                    eng = nc.gpsimd  # earliest-clearing queue -> first seed slice
                elif b == 1 or ch % 2 == 0:
                    eng = nc.sync
                else:
                    eng = nc.scalar
                eng.dma_start(x_t[:, o : o + w], x_d[:, o : o + w])

        g = [{} for _ in range(NB)]

        def seed(b):
            lo = b * BW
            g[b][0] = gpools[b].tile([128, BW], F32, name=f"g0_{b}", tag=f"g{b}")
            for o in range(lo, lo + BW, SEED_W):
                sl = slice(o, o + SEED_W)
                nc.scalar.activation(sqx[:, sl], x_t[:, sl], AF.Square)
                nc.scalar.activation(
                    g[b][0][:, o - lo : o - lo + SEED_W],
                    sqx[:, sl],
                    AF.Exp,
                    scale=omneg[:, 0:1],
                )  # env = exp(-omega^2 x^2 / 2)
                nc.scalar.mul(t_t[:, sl], x_t[:, sl], om2[:, 0:1])
            g[b][1] = gpools[b].tile([128, BW], F32, name=f"g1_{b}", tag=f"g{b}")
            for o in range(0, BW, SEED_W):
                nc.vector.tensor_mul(
                    g[b][1][:, o : o + SEED_W],
                    t_t[:, lo + o : lo + o + SEED_W],
                    g[b][0][:, o : o + SEED_W],
                )

        load_x(0)
        seed(0)
        load_x(1)

        stage = [{} for _ in range(NB)]

        def stage_slot(b, k):
            grp = k // FLUSH
            if grp not in stage[b]:
                stage[b][grp] = spools[b].tile(
                    [128, FLUSH * BW], F16, name=f"st{b}_{grp}", tag=f"st{b}"
                )
            off = (k % FLUSH) * BW
            return stage[b][grp][:, off : off + BW]

        def flush(b, k):
            eng = nc.sync if b == 0 else nc.scalar
            if k >= NJ - FLUSH:
                # epilogue: flush per order so the drain after the last
                # ladder step is one small DMA, not a whole group
                off = (k % FLUSH) * BW
                base = b * NJ * BW + k * BW
                eng.dma_start(
                    out_d[:, base : base + BW],
                    stage[b][k // FLUSH][:, off : off + BW],
                )
            elif k % FLUSH == FLUSH - 1:
                base = b * NJ * BW + (k - FLUSH + 1) * BW
                eng.dma_start(
                    out_d[:, base : base + FLUSH * BW], stage[b][k // FLUSH][:, :]
                )

        def cast(b, k):
            nc.scalar.mul(stage_slot(b, k), g[b][k][:, :], _scale(k))
            flush(b, k)

        def step(b, k, sw):
            # one ladder step, optionally sliced (early steps start before
            # the whole block is seeded)
            c = 2.0 * (k - 1)
            lo = b * BW
            q_t = qpools[b].tile([128, BW], F32, name=f"q{k}_{b}", tag=f"q{b}")
            g[b][k] = gpools[b].tile([128, BW], F32, name=f"g{k}_{b}", tag=f"g{b}")
            for o in range(0, BW, sw):
                nc.vector.tensor_mul(
                    q_t[:, o : o + sw],
                    t_t[:, lo + o : lo + o + sw],
                    g[b][k - 1][:, o : o + sw],
                )
                nc.vector.scalar_tensor_tensor(
                    g[b][k][:, o : o + sw],
                    g[b][k - 2][:, o : o + sw],
                    -c,
                    q_t[:, o : o + sw],
                    ALU.mult,
                    ALU.add,
                )

        # block A runs its first ladder steps while block B is still
        # loading/seeding; every cast is emitted after B's seed chain so
        # the serial ACT queue never delays a ladder dependency
        step(0, 2, SEED_W)
        step(0, 3, SEED_W)
        seed(1)
        step(1, 2, SEED_W)
        step(1, 3, SEED_W)
        for b in range(NB):
            for k0 in range(4):
                cast(b, k0)

        for k in range(4, NJ - 1):
            for b in range(NB):
                step(b, k, BW)
            for b in range(NB):
                cast(b, k)
                g[b].pop(k - 2)

        # k = 31: DVE folds the scaled result straight into the fp16 stage:
        #   r = G_29 * (-c*s31)       (2x tensor_scalar, f32)
        #   stage = (q * s31) + r     (scalar_tensor_tensor, fp16 out)
        # in two half-width pieces so the first half's DMA overlaps the rest
        k = NJ - 1
        c, s31 = 2.0 * (k - 1), _scale(NJ - 1)
        for b in range(NB):
            lo = b * BW
            q_t = qpools[b].tile([128, BW], F32, tag=f"q{b}")
            nc.vector.tensor_mul(q_t[:, :], t_t[:, lo : lo + BW], g[b][k - 1][:, :])
            r_t = qpools[b].tile([128, BW], F32, tag=f"q{b}")
            nc.vector.tensor_scalar_mul(r_t[:, :], g[b][k - 2][:, :], -c * s31)
            slot = stage_slot(b, k)
            h = BW // 4
            base = b * NJ * BW + k * BW
            eng = nc.sync if b == 0 else nc.scalar
            for hh in range(4):
                nc.vector.scalar_tensor_tensor(
                    slot[:, hh * h : (hh + 1) * h],
                    q_t[:, hh * h : (hh + 1) * h],
                    s31,
                    r_t[:, hh * h : (hh + 1) * h],
                    ALU.mult,
                    ALU.add,
                )
                eng.dma_start(
                    out_d[:, base + hh * h : base + (hh + 1) * h],
                    slot[:, hh * h : (hh + 1) * h],
                )

    nc.compile()
    return nc


_CACHED_NC = None

# host descale: psi_k = stored_k * 2^(2k - A_EXP)
_DESCALE = (2.0 ** (2.0 * np.arange(NJ) - A_EXP)).astype(np.float32)


def kernel(x: np.ndarray, omega_kernel: np.ndarray, **run_kwargs) -> np.ndarray:
    global _CACHED_NC
    assert x.shape == (B, NJ, 1) and omega_kernel.shape == (1, 1), (
        x.shape,
        omega_kernel.shape,
    )
    x = np.ascontiguousarray(x, np.float32)
    om = np.ascontiguousarray(
        np.broadcast_to(omega_kernel.astype(np.float32), (128, 1))
    )

    if _CACHED_NC is None:
        _CACHED_NC = _build()
    nc = _CACHED_NC

    in_maps = [
        {
            "x": x[c * BC : (c + 1) * BC].reshape(128, E),
            "om": om,
        }
        for c in range(N_CORES)
    ]
    res = run_bass_kernel_spmd(nc, in_maps, core_ids=list(range(N_CORES)), **run_kwargs)
    full = np.empty((B, NJ, NJ), np.float32)
    out3 = np.empty((128, NJ, E), np.float32)
    for c in range(N_CORES):
        arr = np.asarray(res.results[c]["out"]).view(np.float16)
        for b in range(NB):
            blk = arr[:, b * NJ * BW : (b + 1) * NJ * BW].reshape(128, NJ, BW)
            out3[:, :, b * BW : (b + 1) * BW] = blk
        out3 *= _DESCALE[None, :, None]
        full[c * BC : (c + 1) * BC] = out3.transpose(0, 2, 1).reshape(BC, NJ, NJ)
    if run_kwargs:
        return full, res
    return full


# revision 25
# speedup vs baseline: 1.0101x; 1.0101x over previous
"""Trainium2 Bass kernel for nn_HarmonicOscillatorOrbitals.

out[b, i, j] = exp(-s^2/2) * H_j(s), s = omega * x[b, i, 0], j = 0..31
(physicists' Hermite polynomials), data-parallel over 8 NeuronCores on
the leading batch axis.

Per core (8192 batches = 262144 scalars as [128 partitions, E=2048]):
  env = exp(-(omega*x)^2/2), t = 2*omega*x
  G_0 = env, G_1 = t*env, G_k = t*G_{k-1} - 2(k-1)*G_{k-2}  (= env*H_k)

Engine assignment (measured on HW: DVE+GPSIMD contend on shared SBUF
ports and strictly serialize — GPSIMD is net-negative next to a busy
DVE; DVE+ACT coexist at full speed):
  DVE : whole f32 ladder, 2 ops/step (tensor_mul q = t*G_{k-1};
        scalar_tensor_tensor G_k = (G_{k-2} * -2(k-1)) + q), split in
        two column blocks; order 31 is folded straight into the fp16
        stage by the DVE (prescaled r via 2x tensor_scalar, then a
        scalar_tensor_tensor with fp16 output) so the epilogue never
        waits on an ACT cast.
  ACT : envelope, t, and the output casts: fp16 out with a per-order
        power-of-2 scale 2^(7-2k) folded into the activation-copy so
        the full f32 precision of the ladder survives the 16-bit store
        (simulated end-to-end rel err 3.3e-4 vs the 2e-2 gate).
  DMA : fp16 output = 16 MB/core, flushed per (block, 2 orders); x is
        loaded in 128 KB chunks so the first block seeds early. omega
        arrives host-replicated as [128,1] (no on-chip broadcast).

Host: descale by 2^(2k-7) while unsharding (exact power-of-2).
"""

from contextlib import ExitStack

import numpy as np

import concourse.bacc as bacc
import concourse.mybir as mybir
import concourse.tile as tile
from concourse.bass_utils import run_bass_kernel_spmd

F32 = mybir.dt.float32
F16 = mybir.dt.float16
AF = mybir.ActivationFunctionType
ALU = mybir.AluOpType

NJ = 32          # number of Hermite orders
N_CORES = 8
B = 65536        # full batch
BC = B // N_CORES
E = BC * NJ // 128   # 2048 free elems per partition per core
NB = 2           # column blocks
BW = E // NB
SEED_W = 512     # seed (Square/Exp/t) granularity
XCH = 4          # x-DMA chunks per block
FLUSH = 2        # orders per DMA flush

A_EXP = 7        # stored_k = psi_k * 2^(A_EXP - 2k); max |stored_31| ~ 48.4k


def _scale(k):
    return float(2.0 ** (A_EXP - 2 * k))


def _build():
    nc = bacc.Bacc("TRN2", target_bir_lowering=False, debug=False)
    x_d = nc.dram_tensor("x", [128, E], F32, kind="ExternalInput").ap()
    om_d = nc.dram_tensor("om", [128, 1], F32, kind="ExternalInput").ap()
    # block-major fp16 dump: [128, NB, NJ, BW] flattened
    out_d = nc.dram_tensor("out", [128, NJ * E], F16, kind="ExternalOutput").ap()

    with tile.TileContext(nc) as tc, ExitStack() as ctx:
        cpool = ctx.enter_context(tc.tile_pool(name="const", bufs=1))
        gpools = [
            ctx.enter_context(tc.tile_pool(name=f"g{b}", bufs=6)) for b in range(NB)
        ]
        qpools = [
            ctx.enter_context(tc.tile_pool(name=f"q{b}", bufs=2)) for b in range(NB)
        ]
        spools = [
            ctx.enter_context(tc.tile_pool(name=f"s{b}", bufs=3)) for b in range(NB)
        ]

        om2 = cpool.tile([128, 1], F32)
        nc.scalar.dma_start(om2[:, :], om_d[:, :])
        omneg = cpool.tile([128, 1], F32)
        nc.scalar.activation(omneg[:, :], om2[:, :], AF.Square)  # omega^2
        nc.scalar.mul(omneg[:, :], omneg[:, :], -0.5)            # -omega^2/2
        nc.scalar.mul(om2[:, :], om2[:, :], 2.0)                 # 2*omega

        x_t = cpool.tile([128, E], F32)
        t_t = cpool.tile([128, E], F32)
        sqx = cpool.tile([128, E], F32)

        # block A's x chunks split across the sync and scalar DGE rings so
        # they stream in parallel; block B's loads all go on the sync ring
        # (triggered later, under block A's seed chain) so the serial ACT
        # queue is never blocked by B's triggers
        def load_x(b):
            lo = b * BW
            for ch in range(XCH):
                w = BW // XCH
                o = lo + ch * w
                eng = nc.sync if (b == 1 or ch % 2 == 0) else nc.scalar
                eng.dma_start(x_t[:, o : o + w], x_d[:, o : o + w])

        g = [{} for _ in range(NB)]

        def seed(b):
            lo = b * BW
            g[b][0] = gpools[b].tile([128, BW], F32, name=f"g0_{b}", tag=f"g{b}")
            for o in range(lo, lo + BW, SEED_W):
                sl = slice(o, o + SEED_W)
                nc.scalar.activation(sqx[:, sl], x_t[:, sl], AF.Square)
                nc.scalar.activation(
                    g[b][0][:, o - lo : o - lo + SEED_W],
                    sqx[:, sl],
                    AF.Exp,
                    scale=omneg[:, 0:1],
                )  # env = exp(-omega^2 x^2 / 2)
                nc.scalar.mul(t_t[:, sl], x_t[:, sl], om2[:, 0:1])
            g[b][1] = gpools[b].tile([128, BW], F32, name=f"g1_{b}", tag=f"g{b}")
            for o in range(0, BW, SEED_W):
                nc.vector.tensor_mul(
                    g[b][1][:, o : o + SEED_W],
                    t_t[:, lo + o : lo + o + SEED_W],
                    g[b][0][:, o : o + SEED_W],
                )

        load_x(0)
        seed(0)
        load_x(1)

        stage = [{} for _ in range(NB)]

        def stage_slot(b, k):
            grp = k // FLUSH
            if grp not in stage[b]:
                stage[b][grp] = spools[b].tile(
                    [128, FLUSH * BW], F16, name=f"st{b}_{grp}", tag=f"st{b}"
                )
            off = (k % FLUSH) * BW
            return stage[b][grp][:, off : off + BW]

        def flush(b, k):
            eng = nc.sync if b == 0 else nc.scalar
            if k >= NJ - FLUSH:
                # epilogue: flush per order so the drain after the last
                # ladder step is one small DMA, not a whole group
                off = (k % FLUSH) * BW
                base = b * NJ * BW + k * BW
                eng.dma_start(
                    out_d[:, base : base + BW],
                    stage[b][k // FLUSH][:, off : off + BW],
                )
            elif k % FLUSH == FLUSH - 1:
                base = b * NJ * BW + (k - FLUSH + 1) * BW
                eng.dma_start(
                    out_d[:, base : base + FLUSH * BW], stage[b][k // FLUSH][:, :]
                )

        def cast(b, k):
            nc.scalar.mul(stage_slot(b, k), g[b][k][:, :], _scale(k))
            flush(b, k)

        def step(b, k, sw):
            # one ladder step, optionally sliced (early steps start before
            # the whole block is seeded)
            c = 2.0 * (k - 1)
            lo = b * BW
            q_t = qpools[b].tile([128, BW], F32, name=f"q{k}_{b}", tag=f"q{b}")
            g[b][k] = gpools[b].tile([128, BW], F32, name=f"g{k}_{b}", tag=f"g{b}")
            for o in range(0, BW, sw):
                nc.vector.tensor_mul(
                    q_t[:, o : o + sw],
                    t_t[:, lo + o : lo + o + sw],
                    g[b][k - 1][:, o : o + sw],
                )
                nc.vector.scalar_tensor_tensor(
                    g[b][k][:, o : o + sw],
                    g[b][k - 2][:, o : o + sw],
                    -c,
                    q_t[:, o : o + sw],
                    ALU.mult,
                    ALU.add,
                )

        # block A runs its first ladder steps while block B is still
        # loading/seeding; every cast is emitted after B's seed chain so
        # the serial ACT queue never delays a ladder dependency
        step(0, 2, SEED_W)
        step(0, 3, SEED_W)
        seed(1)
        step(1, 2, SEED_W)
        step(1, 3, SEED_W)
        for b in range(NB):
            for k0 in range(4):
                cast(b, k0)

        for k in range(4, NJ - 1):
            for b in range(NB):
                step(b, k, BW)
            for b in range(NB):
                cast(b, k)
                g[b].pop(k - 2)

        # k = 31: DVE folds the scaled result straight into the fp16 stage:
        #   r = G_29 * (-c*s31)       (2x tensor_scalar, f32)
        #   stage = (q * s31) + r     (scalar_tensor_tensor, fp16 out)
        # in two half-width pieces so the first half's DMA overlaps the rest
        k = NJ - 1
        c, s31 = 2.0 * (k - 1), _scale(NJ - 1)
        for b in range(NB):
            lo = b * BW
            q_t = qpools[b].tile([128, BW], F32, tag=f"q{b}")
            nc.vector.tensor_mul(q_t[:, :], t_t[:, lo : lo + BW], g[b][k - 1][:, :])
            r_t = qpools[b].tile([128, BW], F32, tag=f"q{b}")
            nc.vector.tensor_scalar_mul(r_t[:, :], g[b][k - 2][:, :], -c * s31)
            slot = stage_slot(b, k)
            h = BW // 2
            base = b * NJ * BW + k * BW
            eng = nc.sync if b == 0 else nc.scalar
            for hh in range(2):
                nc.vector.scalar_tensor_tensor(
                    slot[:, hh * h : (hh + 1) * h],
                    q_t[:, hh * h : (hh + 1) * h],
                    s31,
                    r_t[:, hh * h : (hh + 1) * h],
                    ALU.mult,
                    ALU.add,
                )
                eng.dma_start(
                    out_d[:, base + hh * h : base + (hh + 1) * h],
                    slot[:, hh * h : (hh + 1) * h],
                )

    nc.compile()
    return nc


_CACHED_NC = None

# host descale: psi_k = stored_k * 2^(2k - A_EXP)
_DESCALE = (2.0 ** (2.0 * np.arange(NJ) - A_EXP)).astype(np.float32)


def kernel(x: np.ndarray, omega_kernel: np.ndarray, **run_kwargs) -> np.ndarray:
    global _CACHED_NC
    assert x.shape == (B, NJ, 1) and omega_kernel.shape == (1, 1), (
        x.shape,
        omega_kernel.shape,
    )
    x = np.ascontiguousarray(x, np.float32)
    om = np.ascontiguousarray(
        np.broadcast_to(omega_kernel.astype(np.float32), (128, 1))
    )

    if _CACHED_NC is None:
        _CACHED_NC = _build()
    nc = _CACHED_NC

    in_maps = [
        {
            "x": x[c * BC : (c + 1) * BC].reshape(128, E),
            "om": om,
        }
        for c in range(N_CORES)
    ]
    res = run_bass_kernel_spmd(nc, in_maps, core_ids=list(range(N_CORES)), **run_kwargs)
    full = np.empty((B, NJ, NJ), np.float32)
    out3 = np.empty((128, NJ, E), np.float32)
    for c in range(N_CORES):
        arr = np.asarray(res.results[c]["out"]).view(np.float16)
        for b in range(NB):
            blk = arr[:, b * NJ * BW : (b + 1) * NJ * BW].reshape(128, NJ, BW)
            out3[:, :, b * BW : (b + 1) * BW] = blk
        out3 *= _DESCALE[None, :, None]
        full[c * BC : (c + 1) * BC] = out3.transpose(0, 2, 1).reshape(BC, NJ, NJ)
    if run_kwargs:
        return full, res
    return full


# revision 28
# speedup vs baseline: 1.0149x; 1.0048x over previous
"""Trainium2 Bass kernel for nn_HarmonicOscillatorOrbitals.

out[b, i, j] = exp(-s^2/2) * H_j(s), s = omega * x[b, i, 0], j = 0..31
(physicists' Hermite polynomials), data-parallel over 8 NeuronCores on
the leading batch axis.

Per core (8192 batches = 262144 scalars as [128 partitions, E=2048]):
  env = exp(-(omega*x)^2/2), t = 2*omega*x
  G_0 = env, G_1 = t*env, G_k = t*G_{k-1} - 2(k-1)*G_{k-2}  (= env*H_k)

Engine assignment (measured on HW: DVE+GPSIMD contend on shared SBUF
ports and strictly serialize — GPSIMD is net-negative next to a busy
DVE; DVE+ACT coexist at full speed):
  DVE : whole f32 ladder, 2 ops/step (tensor_mul q = t*G_{k-1};
        scalar_tensor_tensor G_k = (G_{k-2} * -2(k-1)) + q), split in
        two column blocks; order 31 is folded straight into the fp16
        stage by the DVE (prescaled r via 2x tensor_scalar, then a
        scalar_tensor_tensor with fp16 output) so the epilogue never
        waits on an ACT cast.
  ACT : envelope, t, and the output casts: fp16 out with a per-order
        power-of-2 scale 2^(7-2k) folded into the activation-copy so
        the full f32 precision of the ladder survives the 16-bit store
        (simulated end-to-end rel err 3.3e-4 vs the 2e-2 gate).
  DMA : fp16 output = 16 MB/core, flushed per (block, 2 orders); x is
        loaded in 128 KB chunks so the first block seeds early. omega
        arrives host-replicated as [128,1] (no on-chip broadcast).

Host: descale by 2^(2k-7) while unsharding (exact power-of-2).
"""

from contextlib import ExitStack

import numpy as np

import concourse.bacc as bacc
import concourse.mybir as mybir
import concourse.tile as tile
from concourse.bass_utils import run_bass_kernel_spmd

F32 = mybir.dt.float32
F16 = mybir.dt.float16
AF = mybir.ActivationFunctionType
ALU = mybir.AluOpType

NJ = 32          # number of Hermite orders
N_CORES = 8
B = 65536        # full batch
BC = B // N_CORES
E = BC * NJ // 128   # 2048 free elems per partition per core
NB = 2           # column blocks
BW = E // NB
SEED_W = 512     # seed (Square/Exp/t) granularity
XCH = 4          # x-DMA chunks per block
FLUSH = 2        # orders per DMA flush

A_EXP = 7        # stored_k = psi_k * 2^(A_EXP - 2k); max |stored_31| ~ 48.4k


def _scale(k):
    return float(2.0 ** (A_EXP - 2 * k))


def _build():
    nc = bacc.Bacc("TRN2", target_bir_lowering=False, debug=False)
    x_d = nc.dram_tensor("x", [128, E], F32, kind="ExternalInput").ap()
    om_d = nc.dram_tensor("om", [128, 1], F32, kind="ExternalInput").ap()
    # block-major fp16 dump: [128, NB, NJ, BW] flattened
    out_d = nc.dram_tensor("out", [128, NJ * E], F16, kind="ExternalOutput").ap()

    with tile.TileContext(nc) as tc, ExitStack() as ctx:
        cpool = ctx.enter_context(tc.tile_pool(name="const", bufs=1))
        gpools = [
            ctx.enter_context(tc.tile_pool(name=f"g{b}", bufs=6)) for b in range(NB)
        ]
        qpools = [
            ctx.enter_context(tc.tile_pool(name=f"q{b}", bufs=2)) for b in range(NB)
        ]
        spools = [
            ctx.enter_context(tc.tile_pool(name=f"s{b}", bufs=3)) for b in range(NB)
        ]

        om2 = cpool.tile([128, 1], F32)
        nc.scalar.dma_start(om2[:, :], om_d[:, :])
        omneg = cpool.tile([128, 1], F32)
        nc.scalar.activation(omneg[:, :], om2[:, :], AF.Square)  # omega^2
        nc.scalar.mul(omneg[:, :], omneg[:, :], -0.5)            # -omega^2/2
        nc.scalar.mul(om2[:, :], om2[:, :], 2.0)                 # 2*omega

        x_t = cpool.tile([128, E], F32)
        t_t = cpool.tile([128, E], F32)
        sqx = cpool.tile([128, E], F32)

        # block A's x chunks split across the sync and scalar DGE rings so
        # they stream in parallel; block B's loads all go on the sync ring
        # (triggered later, under block A's seed chain) so the serial ACT
        # queue is never blocked by B's triggers
        def load_x(b):
            lo = b * BW
            for ch in range(XCH):
                w = BW // XCH
                o = lo + ch * w
                eng = nc.sync if (b == 1 or ch % 2 == 0) else nc.scalar
                eng.dma_start(x_t[:, o : o + w], x_d[:, o : o + w])

        g = [{} for _ in range(NB)]

        def seed(b):
            lo = b * BW
            g[b][0] = gpools[b].tile([128, BW], F32, name=f"g0_{b}", tag=f"g{b}")
            for o in range(lo, lo + BW, SEED_W):
                sl = slice(o, o + SEED_W)
                nc.scalar.activation(sqx[:, sl], x_t[:, sl], AF.Square)
                nc.scalar.activation(
                    g[b][0][:, o - lo : o - lo + SEED_W],
                    sqx[:, sl],
                    AF.Exp,
                    scale=omneg[:, 0:1],
                )  # env = exp(-omega^2 x^2 / 2)
                nc.scalar.mul(t_t[:, sl], x_t[:, sl], om2[:, 0:1])
            g[b][1] = gpools[b].tile([128, BW], F32, name=f"g1_{b}", tag=f"g{b}")
            for o in range(0, BW, SEED_W):
                nc.vector.tensor_mul(
                    g[b][1][:, o : o + SEED_W],
                    t_t[:, lo + o : lo + o + SEED_W],
                    g[b][0][:, o : o + SEED_W],
                )

        load_x(0)
        seed(0)
        load_x(1)

        stage = [{} for _ in range(NB)]

        def stage_slot(b, k):
            grp = k // FLUSH
            if grp not in stage[b]:
                stage[b][grp] = spools[b].tile(
                    [128, FLUSH * BW], F16, name=f"st{b}_{grp}", tag=f"st{b}"
                )
            off = (k % FLUSH) * BW
            return stage[b][grp][:, off : off + BW]

        def flush(b, k):
            eng = nc.sync if b == 0 else nc.scalar
            if k >= NJ - FLUSH:
                # epilogue: flush per order so the drain after the last
                # ladder step is one small DMA, not a whole group
                off = (k % FLUSH) * BW
                base = b * NJ * BW + k * BW
                eng.dma_start(
                    out_d[:, base : base + BW],
                    stage[b][k // FLUSH][:, off : off + BW],
                )
            elif k % FLUSH == FLUSH - 1:
                base = b * NJ * BW + (k - FLUSH + 1) * BW
                eng.dma_start(
                    out_d[:, base : base + FLUSH * BW], stage[b][k // FLUSH][:, :]
                )

        def cast(b, k):
            nc.scalar.mul(stage_slot(b, k), g[b][k][:, :], _scale(k))
            flush(b, k)

        def step(b, k, sw):
            # one ladder step, optionally sliced (early steps start before
            # the whole block is seeded). All products are emitted before
            # all combines so no DVE op directly follows the op it depends
            # on — an independent op in between hides the write-ack latency.
            c = 2.0 * (k - 1)
            lo = b * BW
            q_t = qpools[b].tile([128, BW], F32, name=f"q{k}_{b}", tag=f"q{b}")
            g[b][k] = gpools[b].tile([128, BW], F32, name=f"g{k}_{b}", tag=f"g{b}")
            for o in range(0, BW, sw):
                nc.vector.tensor_mul(
                    q_t[:, o : o + sw],
                    t_t[:, lo + o : lo + o + sw],
                    g[b][k - 1][:, o : o + sw],
                )
            for o in range(0, BW, sw):
                nc.vector.scalar_tensor_tensor(
                    g[b][k][:, o : o + sw],
                    g[b][k - 2][:, o : o + sw],
                    -c,
                    q_t[:, o : o + sw],
                    ALU.mult,
                    ALU.add,
                )

        # block A runs its first ladder steps while block B is still
        # loading/seeding; every cast is emitted after B's seed chain so
        # the serial ACT queue never delays a ladder dependency
        step(0, 2, SEED_W)
        step(0, 3, SEED_W)
        seed(1)
        step(1, 2, SEED_W)
        step(1, 3, SEED_W)
        for b in range(NB):
            for k0 in range(4):
                cast(b, k0)

        for k in range(4, NJ - 1):
            # emit both blocks' products, then both combines: every DVE
            # dependency is >=2 instructions back, so the exec queue
            # pipelines with no exposed ack latency
            c = 2.0 * (k - 1)
            qts = []
            for b in range(NB):
                lo = b * BW
                q_t = qpools[b].tile([128, BW], F32, name=f"q{k}_{b}", tag=f"q{b}")
                g[b][k] = gpools[b].tile([128, BW], F32, name=f"g{k}_{b}", tag=f"g{b}")
                nc.vector.tensor_mul(
                    q_t[:, :], t_t[:, lo : lo + BW], g[b][k - 1][:, :]
                )
                qts.append(q_t)
            for b in range(NB):
                nc.vector.scalar_tensor_tensor(
                    g[b][k][:, :], g[b][k - 2][:, :], -c, qts[b][:, :],
                    ALU.mult, ALU.add,
                )
            for b in range(NB):
                cast(b, k)
                g[b].pop(k - 2)

        # k = 31: DVE folds the scaled result straight into the fp16 stage:
        #   r = G_29 * (-c*s31)       (2x tensor_scalar, f32)
        #   stage = (q * s31) + r     (scalar_tensor_tensor, fp16 out)
        # in two half-width pieces so the first half's DMA overlaps the rest
        k = NJ - 1
        c, s31 = 2.0 * (k - 1), _scale(NJ - 1)
        qrs = []
        for b in range(NB):
            lo = b * BW
            q_t = qpools[b].tile([128, BW], F32, name=f"q31_{b}", tag=f"q{b}")
            nc.vector.tensor_mul(q_t[:, :], t_t[:, lo : lo + BW], g[b][k - 1][:, :])
            r_t = qpools[b].tile([128, BW], F32, name=f"r31_{b}", tag=f"q{b}")
            nc.vector.tensor_scalar_mul(r_t[:, :], g[b][k - 2][:, :], -c * s31)
            qrs.append((q_t, r_t))
        h = BW // 2
        for hh in range(2):
            for b in range(NB):
                q_t, r_t = qrs[b]
                slot = stage_slot(b, k)
                base = b * NJ * BW + k * BW
                eng = nc.sync if b == 0 else nc.scalar
                nc.vector.scalar_tensor_tensor(
                    slot[:, hh * h : (hh + 1) * h],
                    q_t[:, hh * h : (hh + 1) * h],
                    s31,
                    r_t[:, hh * h : (hh + 1) * h],
                    ALU.mult,
                    ALU.add,
                )
                eng.dma_start(
                    out_d[:, base + hh * h : base + (hh + 1) * h],
                    slot[:, hh * h : (hh + 1) * h],
                )

    nc.compile()
    return nc


_CACHED_NC = None

# host descale: psi_k = stored_k * 2^(2k - A_EXP)
_DESCALE = (2.0 ** (2.0 * np.arange(NJ) - A_EXP)).astype(np.float32)


def kernel(x: np.ndarray, omega_kernel: np.ndarray, **run_kwargs) -> np.ndarray:
    global _CACHED_NC
    assert x.shape == (B, NJ, 1) and omega_kernel.shape == (1, 1), (
        x.shape,
        omega_kernel.shape,
    )
    x = np.ascontiguousarray(x, np.float32)
    om = np.ascontiguousarray(
        np.broadcast_to(omega_kernel.astype(np.float32), (128, 1))
    )

    if _CACHED_NC is None:
        _CACHED_NC = _build()
    nc = _CACHED_NC

    in_maps = [
        {
            "x": x[c * BC : (c + 1) * BC].reshape(128, E),
            "om": om,
        }
        for c in range(N_CORES)
    ]
    res = run_bass_kernel_spmd(nc, in_maps, core_ids=list(range(N_CORES)), **run_kwargs)
    full = np.empty((B, NJ, NJ), np.float32)
    out3 = np.empty((128, NJ, E), np.float32)
    for c in range(N_CORES):
        arr = np.asarray(res.results[c]["out"]).view(np.float16)
        for b in range(NB):
            blk = arr[:, b * NJ * BW : (b + 1) * NJ * BW].reshape(128, NJ, BW)
            out3[:, :, b * BW : (b + 1) * BW] = blk
        out3 *= _DESCALE[None, :, None]
        full[c * BC : (c + 1) * BC] = out3.transpose(0, 2, 1).reshape(BC, NJ, NJ)
    if run_kwargs:
        return full, res
    return full


# revision 29
# speedup vs baseline: 1.0175x; 1.0025x over previous
"""Trainium2 Bass kernel for nn_HarmonicOscillatorOrbitals.

out[b, i, j] = exp(-s^2/2) * H_j(s), s = omega * x[b, i, 0], j = 0..31
(physicists' Hermite polynomials), data-parallel over 8 NeuronCores on
the leading batch axis.

Per core (8192 batches = 262144 scalars as [128 partitions, E=2048]):
  env = exp(-(omega*x)^2/2), t = 2*omega*x
  G_0 = env, G_1 = t*env, G_k = t*G_{k-1} - 2(k-1)*G_{k-2}  (= env*H_k)

Engine assignment (measured on HW: DVE+GPSIMD contend on shared SBUF
ports and strictly serialize — GPSIMD is net-negative next to a busy
DVE; DVE+ACT coexist at full speed):
  DVE : whole f32 ladder, 2 ops/step (tensor_mul q = t*G_{k-1};
        scalar_tensor_tensor G_k = (G_{k-2} * -2(k-1)) + q), split in
        two column blocks; order 31 is folded straight into the fp16
        stage by the DVE (prescaled r via 2x tensor_scalar, then a
        scalar_tensor_tensor with fp16 output) so the epilogue never
        waits on an ACT cast.
  ACT : envelope, t, and the output casts: fp16 out with a per-order
        power-of-2 scale 2^(7-2k) folded into the activation-copy so
        the full f32 precision of the ladder survives the 16-bit store
        (simulated end-to-end rel err 3.3e-4 vs the 2e-2 gate).
  DMA : fp16 output = 16 MB/core, flushed per (block, 2 orders); x is
        loaded in 128 KB chunks so the first block seeds early. omega
        arrives host-replicated as [128,1] (no on-chip broadcast).

Host: descale by 2^(2k-7) while unsharding (exact power-of-2).
"""

from contextlib import ExitStack

import numpy as np

import concourse.bacc as bacc
import concourse.mybir as mybir
import concourse.tile as tile
from concourse.bass_utils import run_bass_kernel_spmd

F32 = mybir.dt.float32
F16 = mybir.dt.float16
AF = mybir.ActivationFunctionType
ALU = mybir.AluOpType

NJ = 32          # number of Hermite orders
N_CORES = 8
B = 65536        # full batch
BC = B // N_CORES
E = BC * NJ // 128   # 2048 free elems per partition per core
NB = 2           # column blocks
BW = E // NB
SEED_W = 512     # seed (Square/Exp/t) granularity
XCH = 4          # x-DMA chunks per block
FLUSH = 2        # orders per DMA flush

A_EXP = 7        # stored_k = psi_k * 2^(A_EXP - 2k); max |stored_31| ~ 48.4k


def _scale(k):
    return float(2.0 ** (A_EXP - 2 * k))


def _build():
    nc = bacc.Bacc("TRN2", target_bir_lowering=False, debug=False)
    x_d = nc.dram_tensor("x", [128, E], F32, kind="ExternalInput").ap()
    om_d = nc.dram_tensor("om", [128, 1], F32, kind="ExternalInput").ap()
    # block-major fp16 dump: [128, NB, NJ, BW] flattened
    out_d = nc.dram_tensor("out", [128, NJ * E], F16, kind="ExternalOutput").ap()

    with tile.TileContext(nc) as tc, ExitStack() as ctx:
        cpool = ctx.enter_context(tc.tile_pool(name="const", bufs=1))
        gpools = [
            ctx.enter_context(tc.tile_pool(name=f"g{b}", bufs=6)) for b in range(NB)
        ]
        qpools = [
            ctx.enter_context(tc.tile_pool(name=f"q{b}", bufs=2)) for b in range(NB)
        ]
        spools = [
            ctx.enter_context(tc.tile_pool(name=f"s{b}", bufs=3)) for b in range(NB)
        ]

        om2 = cpool.tile([128, 1], F32)
        nc.scalar.dma_start(om2[:, :], om_d[:, :])
        omneg = cpool.tile([128, 1], F32)
        nc.scalar.activation(omneg[:, :], om2[:, :], AF.Square)  # omega^2
        nc.scalar.mul(omneg[:, :], omneg[:, :], -0.5)            # -omega^2/2
        nc.scalar.mul(om2[:, :], om2[:, :], 2.0)                 # 2*omega

        x_t = cpool.tile([128, E], F32)
        t_t = cpool.tile([128, E], F32)
        sqx = cpool.tile([128, E], F32)

        # block A's x chunks split across the sync and scalar DGE rings so
        # they stream in parallel; block B's loads all go on the sync ring
        # (triggered later, under block A's seed chain) so the serial ACT
        # queue is never blocked by B's triggers
        def load_x(b):
            lo = b * BW
            for ch in range(XCH):
                w = BW // XCH
                o = lo + ch * w
                eng = nc.sync if (b == 1 or ch % 2 == 0) else nc.scalar
                eng.dma_start(x_t[:, o : o + w], x_d[:, o : o + w])

        g = [{} for _ in range(NB)]

        def seed(b):
            lo = b * BW
            g[b][0] = gpools[b].tile([128, BW], F32, name=f"g0_{b}", tag=f"g{b}")
            for o in range(lo, lo + BW, SEED_W):
                sl = slice(o, o + SEED_W)
                # t on the (idle-at-startup) DVE via 2x tensor_scalar with
                # the omega AP, shortening ACT's serial seed chain to
                # Square -> Exp only
                nc.vector.tensor_scalar_mul(t_t[:, sl], x_t[:, sl], om2[:, 0:1])
                nc.scalar.activation(sqx[:, sl], x_t[:, sl], AF.Square)
                nc.scalar.activation(
                    g[b][0][:, o - lo : o - lo + SEED_W],
                    sqx[:, sl],
                    AF.Exp,
                    scale=omneg[:, 0:1],
                )  # env = exp(-omega^2 x^2 / 2)
            g[b][1] = gpools[b].tile([128, BW], F32, name=f"g1_{b}", tag=f"g{b}")
            for o in range(0, BW, SEED_W):
                nc.vector.tensor_mul(
                    g[b][1][:, o : o + SEED_W],
                    t_t[:, lo + o : lo + o + SEED_W],
                    g[b][0][:, o : o + SEED_W],
                )

        load_x(0)
        seed(0)
        load_x(1)

        stage = [{} for _ in range(NB)]

        def stage_slot(b, k):
            grp = k // FLUSH
            if grp not in stage[b]:
                stage[b][grp] = spools[b].tile(
                    [128, FLUSH * BW], F16, name=f"st{b}_{grp}", tag=f"st{b}"
                )
            off = (k % FLUSH) * BW
            return stage[b][grp][:, off : off + BW]

        def flush(b, k):
            eng = nc.sync if b == 0 else nc.scalar
            if k >= NJ - FLUSH:
                # epilogue: flush per order so the drain after the last
                # ladder step is one small DMA, not a whole group
                off = (k % FLUSH) * BW
                base = b * NJ * BW + k * BW
                eng.dma_start(
                    out_d[:, base : base + BW],
                    stage[b][k // FLUSH][:, off : off + BW],
                )
            elif k % FLUSH == FLUSH - 1:
                base = b * NJ * BW + (k - FLUSH + 1) * BW
                eng.dma_start(
                    out_d[:, base : base + FLUSH * BW], stage[b][k // FLUSH][:, :]
                )

        def cast(b, k):
            nc.scalar.mul(stage_slot(b, k), g[b][k][:, :], _scale(k))
            flush(b, k)

        def step(b, k, sw):
            # one ladder step, optionally sliced (early steps start before
            # the whole block is seeded). All products are emitted before
            # all combines so no DVE op directly follows the op it depends
            # on — an independent op in between hides the write-ack latency.
            c = 2.0 * (k - 1)
            lo = b * BW
            q_t = qpools[b].tile([128, BW], F32, name=f"q{k}_{b}", tag=f"q{b}")
            g[b][k] = gpools[b].tile([128, BW], F32, name=f"g{k}_{b}", tag=f"g{b}")
            for o in range(0, BW, sw):
                nc.vector.tensor_mul(
                    q_t[:, o : o + sw],
                    t_t[:, lo + o : lo + o + sw],
                    g[b][k - 1][:, o : o + sw],
                )
            for o in range(0, BW, sw):
                nc.vector.scalar_tensor_tensor(
                    g[b][k][:, o : o + sw],
                    g[b][k - 2][:, o : o + sw],
                    -c,
                    q_t[:, o : o + sw],
                    ALU.mult,
                    ALU.add,
                )

        # block A runs its first ladder steps while block B is still
        # loading/seeding; every cast is emitted after B's seed chain so
        # the serial ACT queue never delays a ladder dependency
        step(0, 2, SEED_W)
        step(0, 3, SEED_W)
        seed(1)
        step(1, 2, SEED_W)
        step(1, 3, SEED_W)
        for b in range(NB):
            for k0 in range(4):
                cast(b, k0)

        for k in range(4, NJ - 1):
            # emit both blocks' products, then both combines: every DVE
            # dependency is >=2 instructions back, so the exec queue
            # pipelines with no exposed ack latency
            c = 2.0 * (k - 1)
            qts = []
            for b in range(NB):
                lo = b * BW
                q_t = qpools[b].tile([128, BW], F32, name=f"q{k}_{b}", tag=f"q{b}")
                g[b][k] = gpools[b].tile([128, BW], F32, name=f"g{k}_{b}", tag=f"g{b}")
                nc.vector.tensor_mul(
                    q_t[:, :], t_t[:, lo : lo + BW], g[b][k - 1][:, :]
                )
                qts.append(q_t)
            for b in range(NB):
                nc.vector.scalar_tensor_tensor(
                    g[b][k][:, :], g[b][k - 2][:, :], -c, qts[b][:, :],
                    ALU.mult, ALU.add,
                )
            for b in range(NB):
                cast(b, k)
                g[b].pop(k - 2)

        # k = 31: DVE folds the scaled result straight into the fp16 stage:
        #   r = G_29 * (-c*s31)       (2x tensor_scalar, f32)
        #   stage = (q * s31) + r     (scalar_tensor_tensor, fp16 out)
        # in two half-width pieces so the first half's DMA overlaps the rest
        k = NJ - 1
        c, s31 = 2.0 * (k - 1), _scale(NJ - 1)
        qrs = []
        for b in range(NB):
            lo = b * BW
            q_t = qpools[b].tile([128, BW], F32, name=f"q31_{b}", tag=f"q{b}")
            nc.vector.tensor_mul(q_t[:, :], t_t[:, lo : lo + BW], g[b][k - 1][:, :])
            r_t = qpools[b].tile([128, BW], F32, name=f"r31_{b}", tag=f"q{b}")
            nc.vector.tensor_scalar_mul(r_t[:, :], g[b][k - 2][:, :], -c * s31)
            qrs.append((q_t, r_t))
        h = BW // 2
        for hh in range(2):
            for b in range(NB):
                q_t, r_t = qrs[b]
                slot = stage_slot(b, k)
                base = b * NJ * BW + k * BW
                eng = nc.sync if b == 0 else nc.scalar
                nc.vector.scalar_tensor_tensor(
                    slot[:, hh * h : (hh + 1) * h],
                    q_t[:, hh * h : (hh + 1) * h],
                    s31,
                    r_t[:, hh * h : (hh + 1) * h],
                    ALU.mult,
                    ALU.add,
                )
                eng.dma_start(
                    out_d[:, base + hh * h : base + (hh + 1) * h],
                    slot[:, hh * h : (hh + 1) * h],
                )

    nc.compile()
    return nc


_CACHED_NC = None

# host descale: psi_k = stored_k * 2^(2k - A_EXP)
_DESCALE = (2.0 ** (2.0 * np.arange(NJ) - A_EXP)).astype(np.float32)


def kernel(x: np.ndarray, omega_kernel: np.ndarray, **run_kwargs) -> np.ndarray:
    global _CACHED_NC
    assert x.shape == (B, NJ, 1) and omega_kernel.shape == (1, 1), (
        x.shape,
        omega_kernel.shape,
    )
    x = np.ascontiguousarray(x, np.float32)
    om = np.ascontiguousarray(
        np.broadcast_to(omega_kernel.astype(np.float32), (128, 1))
    )

    if _CACHED_NC is None:
        _CACHED_NC = _build()
    nc = _CACHED_NC

    in_maps = [
        {
            "x": x[c * BC : (c + 1) * BC].reshape(128, E),
            "om": om,
        }
        for c in range(N_CORES)
    ]
    res = run_bass_kernel_spmd(nc, in_maps, core_ids=list(range(N_CORES)), **run_kwargs)
    full = np.empty((B, NJ, NJ), np.float32)
    out3 = np.empty((128, NJ, E), np.float32)
    for c in range(N_CORES):
        arr = np.asarray(res.results[c]["out"]).view(np.float16)
        for b in range(NB):
            blk = arr[:, b * NJ * BW : (b + 1) * NJ * BW].reshape(128, NJ, BW)
            out3[:, :, b * BW : (b + 1) * BW] = blk
        out3 *= _DESCALE[None, :, None]
        full[c * BC : (c + 1) * BC] = out3.transpose(0, 2, 1).reshape(BC, NJ, NJ)
    if run_kwargs:
        return full, res
    return full


# revision 30
# speedup vs baseline: 1.0236x; 1.0060x over previous
"""Trainium2 Bass kernel for nn_HarmonicOscillatorOrbitals.

out[b, i, j] = exp(-s^2/2) * H_j(s), s = omega * x[b, i, 0], j = 0..31
(physicists' Hermite polynomials), data-parallel over 8 NeuronCores on
the leading batch axis.

Per core (8192 batches = 262144 scalars as [128 partitions, E=2048]):
  env = exp(-(omega*x)^2/2), t = 2*omega*x
  G_0 = env, G_1 = t*env, G_k = t*G_{k-1} - 2(k-1)*G_{k-2}  (= env*H_k)

Engine assignment (measured on HW: DVE+GPSIMD contend on shared SBUF
ports and strictly serialize — GPSIMD is net-negative next to a busy
DVE; DVE+ACT coexist at full speed):
  DVE : whole f32 ladder, 2 ops/step (tensor_mul q = t*G_{k-1};
        scalar_tensor_tensor G_k = (G_{k-2} * -2(k-1)) + q), split in
        two column blocks; order 31 is folded straight into the fp16
        stage by the DVE (prescaled r via 2x tensor_scalar, then a
        scalar_tensor_tensor with fp16 output) so the epilogue never
        waits on an ACT cast.
  ACT : envelope, t, and the output casts: fp16 out with a per-order
        power-of-2 scale 2^(7-2k) folded into the activation-copy so
        the full f32 precision of the ladder survives the 16-bit store
        (simulated end-to-end rel err 3.3e-4 vs the 2e-2 gate).
  DMA : fp16 output = 16 MB/core, flushed per (block, 2 orders); x is
        loaded in 128 KB chunks so the first block seeds early. omega
        arrives host-replicated as [128,1] (no on-chip broadcast).

Host: descale by 2^(2k-7) while unsharding (exact power-of-2).
"""

from contextlib import ExitStack

import numpy as np

import concourse.bacc as bacc
import concourse.mybir as mybir
import concourse.tile as tile
from concourse.bass_utils import run_bass_kernel_spmd

F32 = mybir.dt.float32
F16 = mybir.dt.float16
AF = mybir.ActivationFunctionType
ALU = mybir.AluOpType

NJ = 32          # number of Hermite orders
N_CORES = 8
B = 65536        # full batch
BC = B // N_CORES
E = BC * NJ // 128   # 2048 free elems per partition per core
NB = 2           # column blocks
BW = E // NB
SEED_W = 512     # seed (Square/Exp/t) granularity
XCH = 4          # x-DMA chunks per block
FLUSH = 2        # orders per DMA flush

A_EXP = 7        # stored_k = psi_k * 2^(A_EXP - 2k); max |stored_31| ~ 48.4k


def _scale(k):
    return float(2.0 ** (A_EXP - 2 * k))


def _build():
    nc = bacc.Bacc("TRN2", target_bir_lowering=False, debug=False)
    x_d = nc.dram_tensor("x", [128, E], F32, kind="ExternalInput").ap()
    om_d = nc.dram_tensor("om", [128, 1], F32, kind="ExternalInput").ap()
    # block-major fp16 dump: [128, NB, NJ, BW] flattened
    out_d = nc.dram_tensor("out", [128, NJ * E], F16, kind="ExternalOutput").ap()

    with tile.TileContext(nc) as tc, ExitStack() as ctx:
        cpool = ctx.enter_context(tc.tile_pool(name="const", bufs=1))
        gpools = [
            ctx.enter_context(tc.tile_pool(name=f"g{b}", bufs=6)) for b in range(NB)
        ]
        qpools = [
            ctx.enter_context(tc.tile_pool(name=f"q{b}", bufs=2)) for b in range(NB)
        ]
        spools = [
            ctx.enter_context(tc.tile_pool(name=f"s{b}", bufs=3)) for b in range(NB)
        ]

        om2 = cpool.tile([128, 1], F32)
        nc.scalar.dma_start(om2[:, :], om_d[:, :])
        omneg = cpool.tile([128, 1], F32)
        nc.scalar.activation(omneg[:, :], om2[:, :], AF.Square)  # omega^2
        nc.scalar.mul(omneg[:, :], omneg[:, :], -0.5)            # -omega^2/2
        nc.scalar.mul(om2[:, :], om2[:, :], 2.0)                 # 2*omega

        x_t = cpool.tile([128, E], F32)
        t_t = cpool.tile([128, E], F32)
        sqx = cpool.tile([128, E], F32)

        # block A's x chunks split across the sync and scalar DGE rings so
        # they stream in parallel; block B's loads all go on the sync ring
        # (triggered later, under block A's seed chain) so the serial ACT
        # queue is never blocked by B's triggers
        def load_x(b):
            lo = b * BW
            for ch in range(XCH):
                w = BW // XCH
                o = lo + ch * w
                eng = nc.sync if (b == 1 or ch % 2 == 0) else nc.scalar
                eng.dma_start(x_t[:, o : o + w], x_d[:, o : o + w])

        g = [{} for _ in range(NB)]

        def seed(b):
            lo = b * BW
            g[b][0] = gpools[b].tile([128, BW], F32, name=f"g0_{b}", tag=f"g{b}")
            for o in range(lo, lo + BW, SEED_W):
                sl = slice(o, o + SEED_W)
                # t (and, for block A, x^2) on the idle-at-startup DVE,
                # shortening ACT's serial seed chain on the critical path
                # to the single Exp link
                nc.vector.tensor_scalar_mul(t_t[:, sl], x_t[:, sl], om2[:, 0:1])
                if b == 0:
                    nc.vector.tensor_mul(sqx[:, sl], x_t[:, sl], x_t[:, sl])
                else:
                    nc.scalar.activation(sqx[:, sl], x_t[:, sl], AF.Square)
                nc.scalar.activation(
                    g[b][0][:, o - lo : o - lo + SEED_W],
                    sqx[:, sl],
                    AF.Exp,
                    scale=omneg[:, 0:1],
                )  # env = exp(-omega^2 x^2 / 2)
            g[b][1] = gpools[b].tile([128, BW], F32, name=f"g1_{b}", tag=f"g{b}")
            for o in range(0, BW, SEED_W):
                nc.vector.tensor_mul(
                    g[b][1][:, o : o + SEED_W],
                    t_t[:, lo + o : lo + o + SEED_W],
                    g[b][0][:, o : o + SEED_W],
                )

        load_x(0)
        seed(0)
        load_x(1)

        stage = [{} for _ in range(NB)]

        def stage_slot(b, k):
            grp = k // FLUSH
            if grp not in stage[b]:
                stage[b][grp] = spools[b].tile(
                    [128, FLUSH * BW], F16, name=f"st{b}_{grp}", tag=f"st{b}"
                )
            off = (k % FLUSH) * BW
            return stage[b][grp][:, off : off + BW]

        def flush(b, k):
            eng = nc.sync if b == 0 else nc.scalar
            if k >= NJ - FLUSH:
                # epilogue: flush per order so the drain after the last
                # ladder step is one small DMA, not a whole group
                off = (k % FLUSH) * BW
                base = b * NJ * BW + k * BW
                eng.dma_start(
                    out_d[:, base : base + BW],
                    stage[b][k // FLUSH][:, off : off + BW],
                )
            elif k % FLUSH == FLUSH - 1:
                base = b * NJ * BW + (k - FLUSH + 1) * BW
                eng.dma_start(
                    out_d[:, base : base + FLUSH * BW], stage[b][k // FLUSH][:, :]
                )

        def cast(b, k):
            nc.scalar.mul(stage_slot(b, k), g[b][k][:, :], _scale(k))
            flush(b, k)

        def step(b, k, sw):
            # one ladder step, optionally sliced (early steps start before
            # the whole block is seeded). All products are emitted before
            # all combines so no DVE op directly follows the op it depends
            # on — an independent op in between hides the write-ack latency.
            c = 2.0 * (k - 1)
            lo = b * BW
            q_t = qpools[b].tile([128, BW], F32, name=f"q{k}_{b}", tag=f"q{b}")
            g[b][k] = gpools[b].tile([128, BW], F32, name=f"g{k}_{b}", tag=f"g{b}")
            for o in range(0, BW, sw):
                nc.vector.tensor_mul(
                    q_t[:, o : o + sw],
                    t_t[:, lo + o : lo + o + sw],
                    g[b][k - 1][:, o : o + sw],
                )
            for o in range(0, BW, sw):
                nc.vector.scalar_tensor_tensor(
                    g[b][k][:, o : o + sw],
                    g[b][k - 2][:, o : o + sw],
                    -c,
                    q_t[:, o : o + sw],
                    ALU.mult,
                    ALU.add,
                )

        # block A runs its first ladder steps while block B is still
        # loading/seeding; every cast is emitted after B's seed chain so
        # the serial ACT queue never delays a ladder dependency
        step(0, 2, SEED_W)
        step(0, 3, SEED_W)
        seed(1)
        step(1, 2, SEED_W)
        step(1, 3, SEED_W)
        for b in range(NB):
            for k0 in range(4):
                cast(b, k0)

        for k in range(4, NJ - 1):
            # emit both blocks' products, then both combines: every DVE
            # dependency is >=2 instructions back, so the exec queue
            # pipelines with no exposed ack latency
            c = 2.0 * (k - 1)
            qts = []
            for b in range(NB):
                lo = b * BW
                q_t = qpools[b].tile([128, BW], F32, name=f"q{k}_{b}", tag=f"q{b}")
                g[b][k] = gpools[b].tile([128, BW], F32, name=f"g{k}_{b}", tag=f"g{b}")
                nc.vector.tensor_mul(
                    q_t[:, :], t_t[:, lo : lo + BW], g[b][k - 1][:, :]
                )
                qts.append(q_t)
            for b in range(NB):
                nc.vector.scalar_tensor_tensor(
                    g[b][k][:, :], g[b][k - 2][:, :], -c, qts[b][:, :],
                    ALU.mult, ALU.add,
                )
            for b in range(NB):
                cast(b, k)
                g[b].pop(k - 2)

        # k = 31: DVE folds the scaled result straight into the fp16 stage:
        #   r = G_29 * (-c*s31)       (2x tensor_scalar, f32)
        #   stage = (q * s31) + r     (scalar_tensor_tensor, fp16 out)
        # in two half-width pieces so the first half's DMA overlaps the rest
        k = NJ - 1
        c, s31 = 2.0 * (k - 1), _scale(NJ - 1)
        qrs = []
        for b in range(NB):
            lo = b * BW
            q_t = qpools[b].tile([128, BW], F32, name=f"q31_{b}", tag=f"q{b}")
            nc.vector.tensor_mul(q_t[:, :], t_t[:, lo : lo + BW], g[b][k - 1][:, :])
            r_t = qpools[b].tile([128, BW], F32, name=f"r31_{b}", tag=f"q{b}")
            nc.vector.tensor_scalar_mul(r_t[:, :], g[b][k - 2][:, :], -c * s31)
            qrs.append((q_t, r_t))
        h = BW // 2
        for hh in range(2):
            for b in range(NB):
                q_t, r_t = qrs[b]
                slot = stage_slot(b, k)
                base = b * NJ * BW + k * BW
                eng = nc.sync if b == 0 else nc.scalar
                nc.vector.scalar_tensor_tensor(
                    slot[:, hh * h : (hh + 1) * h],
                    q_t[:, hh * h : (hh + 1) * h],
                    s31,
                    r_t[:, hh * h : (hh + 1) * h],
                    ALU.mult,
                    ALU.add,
                )
                eng.dma_start(
                    out_d[:, base + hh * h : base + (hh + 1) * h],
                    slot[:, hh * h : (hh + 1) * h],
                )

    nc.compile()
    return nc


_CACHED_NC = None

# host descale: psi_k = stored_k * 2^(2k - A_EXP)
_DESCALE = (2.0 ** (2.0 * np.arange(NJ) - A_EXP)).astype(np.float32)


def kernel(x: np.ndarray, omega_kernel: np.ndarray, **run_kwargs) -> np.ndarray:
    global _CACHED_NC
    assert x.shape == (B, NJ, 1) and omega_kernel.shape == (1, 1), (
        x.shape,
        omega_kernel.shape,
    )
    x = np.ascontiguousarray(x, np.float32)
    om = np.ascontiguousarray(
        np.broadcast_to(omega_kernel.astype(np.float32), (128, 1))
    )

    if _CACHED_NC is None:
        _CACHED_NC = _build()
    nc = _CACHED_NC

    in_maps = [
        {
            "x": x[c * BC : (c + 1) * BC].reshape(128, E),
            "om": om,
        }
        for c in range(N_CORES)
    ]
    res = run_bass_kernel_spmd(nc, in_maps, core_ids=list(range(N_CORES)), **run_kwargs)
    full = np.empty((B, NJ, NJ), np.float32)
    out3 = np.empty((128, NJ, E), np.float32)
    for c in range(N_CORES):
        arr = np.asarray(res.results[c]["out"]).view(np.float16)
        for b in range(NB):
            blk = arr[:, b * NJ * BW : (b + 1) * NJ * BW].reshape(128, NJ, BW)
            out3[:, :, b * BW : (b + 1) * BW] = blk
        out3 *= _DESCALE[None, :, None]
        full[c * BC : (c + 1) * BC] = out3.transpose(0, 2, 1).reshape(BC, NJ, NJ)
    if run_kwargs:
        return full, res
    return full
